# revision 2
# baseline (speedup 1.0000x reference)
"""MoD-router FFN kernel for 8 TRN2 NeuronCores (self-contained).

Math note: the reference applies softmax over a size-1 axis, which yields
all-ones scores for ANY input; jax.lax.top_k is stable, so the selected
token indices are always [0..NUM_TOKENS) per batch row. The router weights
(Wp, bp) therefore cannot affect the output, and the kernel computes

    out = gelu_tanh(x[:, :2048, :] @ W1 + b1) @ W2 + b2

Sharding: data-parallel over the 4*2048 = 8192 selected token rows ->
1024 rows per core. Each core runs a fused transposed FFN:
  H^T = gelu(W1^T @ X^T + b1)   (per F-block of 512, kept in SBUF)
  out^T += W2_blk^T @ H^T_blk   (accumulated in SBUF fp32)
Matmuls run in bf16 (same 1 col/cycle PE rate as fp32r but FWL halves
LDWEIGHTS and DMA bytes drop 2x; quantization adds ~3e-3 rel err, well
under the 2e-2 gate). PSUM accumulation stays fp32.
"""

import numpy as np

B, S, D, F = 4, 4096, 2048, 8192
NUM_TOKENS = 2048
NCORES = 8
ROWS = (B * NUM_TOKENS) // NCORES     # 1024 rows per core
P = 128
KT = D // P                           # 16 k-subtiles over D
FT = F // P                           # 64 f-tiles
FB = 16                               # F-blocks of 512
FSUB = 4                              # f-subtiles per block
DT = D // P                           # 16 d-tiles
NCH = ROWS // 512                     # 2 row chunks of 512
KS2 = 4                               # k-subtiles per F-block in FFN2

_CACHE = {}


def _build():
    import concourse.bass as bass
    import concourse.mybir as mybir
    import concourse.tile as tile
    from concourse import bacc

    f32 = mybir.dt.float32
    bf16 = mybir.dt.bfloat16

    nc = bacc.Bacc()
    xt = nc.declare_dram_parameter("xt", [KT, P, ROWS], bf16, isOutput=False)
    w1 = nc.declare_dram_parameter("w1", [FT, P, KT * P], bf16, isOutput=False)
    w2 = nc.declare_dram_parameter("w2", [FB, DT, P, KS2 * P], bf16, isOutput=False)
    b1 = nc.declare_dram_parameter("b1", [P, FT], f32, isOutput=False)
    b2 = nc.declare_dram_parameter("b2", [P, DT], f32, isOutput=False)
    out = nc.declare_dram_parameter("out", [DT, P, ROWS], f32, isOutput=True)

    with tile.TileContext(nc) as tc:
        with (
            tc.tile_pool(name="resident", bufs=1) as res_pool,
            tc.tile_pool(name="w1p", bufs=6) as w1p,
            tc.tile_pool(name="w2p", bufs=8) as w2p,
            tc.tile_pool(name="htp", bufs=8) as htp,
            tc.tile_pool(name="ps1", bufs=4, space="PSUM") as ps1,
            tc.tile_pool(name="ps2", bufs=4, space="PSUM") as ps2,
        ):
            xt_sb = [res_pool.tile([P, ROWS], bf16, name=f"xt{k}") for k in range(KT)]
            w1_warm = [w1p.tile([P, KT * P], bf16, name=f"w1t_{ft}", tag="w1t")
                       for ft in range(FSUB)]
            b1_sb = res_pool.tile([P, FT], f32, name="b1sb")
            b2_sb = res_pool.tile([P, DT], f32, name="b2sb")
            oacc = [res_pool.tile([P, ROWS], f32, name=f"oacc{d}") for d in range(DT)]

            # Startup DMAs, first-needed-first. Warmup half 0 (fs 0/1) needs
            # w1 ft0/ft1 low-k slices + xt[0..] in k order; half 1 needs
            # ft2/ft3. All queues share HBM BW, so issue order ~= land order.
            HK = 8 * P                      # half a w1 tile (k 0..7)
            nc.sync.dma_start(out=b1_sb[:], in_=b1[:])
            nc.sync.dma_start(out=b2_sb[:], in_=b2[:])
            for ft in range(FSUB):
                nc.sync.dma_start(out=w1_warm[ft][:, :HK], in_=w1[ft, :, :HK])
            for k in range(2):
                for n in range(NCH):
                    nc.sync.dma_start(out=xt_sb[k][:, n * 512:(n + 1) * 512],
                                      in_=xt[k, :, n * 512:(n + 1) * 512])
            for ft in range(FSUB):
                nc.sync.dma_start(out=w1_warm[ft][:, HK:], in_=w1[ft, :, HK:])
            for k in range(2, KT):
                nc.sync.dma_start(out=xt_sb[k][:], in_=xt[k])

            for fb in range(FB):
                ht = []
                if fb == 0:
                    # warmup block: k-outer over 4 concurrent psum chains
                    # (2 f-subtiles x 2 row chunks per pass) so matmuls start
                    # as soon as xt_sb[k] lands instead of waiting for all XT.
                    for fs in range(FSUB):
                        ht.append(htp.tile([P, ROWS], bf16, name=f"ht_{fs}", tag="ht"))
                    for half in range(2):
                        chains = [(half * 2 + i, n) for i in range(2) for n in range(NCH)]
                        psums = {
                            c: ps1.tile([P, 512], f32, name=f"ps1w_{c[0]}_{c[1]}", tag="ps1")
                            for c in chains
                        }
                        for k in range(KT):
                            for fs, n in chains:
                                nc.tensor.matmul(
                                    psums[(fs, n)][:],
                                    w1_warm[fs][:, k * P:(k + 1) * P],
                                    xt_sb[k][:, n * 512:(n + 1) * 512],
                                    start=(k == 0), stop=(k == KT - 1),
                                )
                        for fs, n in chains:
                            nc.scalar.activation(
                                ht[fs][:, n * 512:(n + 1) * 512], psums[(fs, n)][:],
                                mybir.ActivationFunctionType.Gelu_apprx_tanh,
                                bias=b1_sb[:, fs:fs + 1],
                            )
                else:
                    for fs in range(FSUB):
                        ft = fb * FSUB + fs
                        w1_sb = w1p.tile([P, KT * P], bf16, name=f"w1t_{ft}", tag="w1t")
                        nc.sync.dma_start(out=w1_sb[:], in_=w1[ft])
                        ht_t = htp.tile([P, ROWS], bf16, name=f"ht_{ft}", tag="ht")
                        psums = [ps1.tile([P, 512], f32, name=f"ps1_{ft}_{n}", tag="ps1")
                                 for n in range(NCH)]
                        # n innermost: consecutive matmuls share the
                        # stationary w1 k-slice (one LDWEIGHTS per pair).
                        for k in range(KT):
                            for n in range(NCH):
                                nc.tensor.matmul(
                                    psums[n][:],
                                    w1_sb[:, k * P:(k + 1) * P],
                                    xt_sb[k][:, n * 512:(n + 1) * 512],
                                    start=(k == 0), stop=(k == KT - 1),
                                )
                        for n in range(NCH):
                            nc.scalar.activation(
                                ht_t[:, n * 512:(n + 1) * 512], psums[n][:],
                                mybir.ActivationFunctionType.Gelu_apprx_tanh,
                                bias=b1_sb[:, ft:ft + 1],
                            )
                        ht.append(ht_t)

                for d in range(DT):
                    w2_sb = w2p.tile([P, KS2 * P], bf16, name=f"w2t_{fb}_{d}", tag="w2t")
                    nc.sync.dma_start(out=w2_sb[:], in_=w2[fb, d])
                    psums2 = [ps2.tile([P, 512], f32, name=f"ps2_{fb}_{d}_{n}", tag="ps2")
                              for n in range(NCH)]
                    for ks in range(KS2):
                        for n in range(NCH):
                            nc.tensor.matmul(
                                psums2[n][:],
                                w2_sb[:, ks * P:(ks + 1) * P],
                                ht[ks][:, n * 512:(n + 1) * 512],
                                start=(ks == 0), stop=(ks == KS2 - 1),
                            )
                    for n in range(NCH):
                        sl = slice(n * 512, (n + 1) * 512)
                        if fb == 0:
                            # first block: write psum + broadcast b2 directly
                            nc.scalar.activation(
                                oacc[d][:, sl], psums2[n][:],
                                mybir.ActivationFunctionType.Identity,
                                bias=b2_sb[:, d:d + 1],
                            )
                        else:
                            nc.vector.tensor_add(
                                oacc[d][:, sl], oacc[d][:, sl], psums2[n][:])
                        if fb == FB - 1:
                            # stream the finished output chunk while the
                            # remaining d-tiles still compute
                            nc.sync.dma_start(out=out[d, :, sl], in_=oacc[d][:, sl])

    nc.compile()
    return nc


def _get_nc():
    if "nc" not in _CACHE:
        _CACHE["nc"] = _build()
    return _CACHE["nc"]


def _prep_in_maps(x, W1, b1, W2, b2):
    import ml_dtypes

    bf = ml_dtypes.bfloat16
    x = np.asarray(x, dtype=np.float32)
    W1 = np.asarray(W1, dtype=np.float32)
    W2 = np.asarray(W2, dtype=np.float32)
    b1 = np.asarray(b1, dtype=np.float32)
    b2 = np.asarray(b2, dtype=np.float32)

    xs = x[:, :NUM_TOKENS, :].reshape(B * NUM_TOKENS, D)         # [8192, 2048]
    w1h = np.ascontiguousarray(
        W1.reshape(KT, P, FT, P).transpose(2, 1, 0, 3)
    ).reshape(FT, P, KT * P).astype(bf)                          # [ft, p, k*c]
    w2h = np.ascontiguousarray(
        W2.reshape(FB, KS2, P, DT, P).transpose(0, 3, 2, 1, 4)
    ).reshape(FB, DT, P, KS2 * P).astype(bf)                     # [fb, d, p, ks*c]
    b1h = np.ascontiguousarray(b1.reshape(FT, P).T)              # [p, ft]
    b2h = np.ascontiguousarray(b2.reshape(DT, P).T)              # [p, d]

    in_maps = []
    for c in range(NCORES):
        xc = xs[c * ROWS:(c + 1) * ROWS]                         # [1024, 2048]
        xth = np.ascontiguousarray(xc.T.reshape(KT, P, ROWS)).astype(bf)
        in_maps.append({"xt": xth, "w1": w1h, "w2": w2h, "b1": b1h, "b2": b2h})
    return in_maps


def _gather(res):
    out = np.empty((B * NUM_TOKENS, D), dtype=np.float32)
    for c in range(NCORES):
        oc = res.results[c]["out"]                               # [d, p, n]
        out[c * ROWS:(c + 1) * ROWS] = oc.reshape(D, ROWS).T
    return out.reshape(B, NUM_TOKENS, D)


def kernel(x, Wp, bp, W1, b1, W2, b2, **_unused):
    from concourse.bass_utils import run_bass_kernel_spmd

    in_maps = _prep_in_maps(x, W1, b1, W2, b2)
    nc = _get_nc()
    res = run_bass_kernel_spmd(nc, in_maps, list(range(NCORES)))
    return _gather(res)


# revision 6
# speedup vs baseline: 1.0105x; 1.0105x over previous
"""MoD-router FFN kernel for 8 TRN2 NeuronCores (self-contained).

Math note: the reference applies softmax over a size-1 axis, which yields
all-ones scores for ANY input; jax.lax.top_k is stable, so the selected
token indices are always [0..NUM_TOKENS) per batch row. The router weights
(Wp, bp) therefore cannot affect the output, and the kernel computes

    out = gelu_tanh(x[:, :2048, :] @ W1 + b1) @ W2 + b2

Sharding: data-parallel over the 4*2048 = 8192 selected token rows ->
1024 rows per core. Each core runs a fused transposed FFN:
  H^T = gelu(W1^T @ X^T + b1)   (per F-block of 512, kept in SBUF)
  out^T += W2_blk^T @ H^T_blk   (accumulated in SBUF fp32)
Matmuls run in bf16 (same 1 col/cycle PE rate as fp32r but FWL hides
LDWEIGHTS and DMA bytes drop 2x; quantization adds ~3.4e-3 rel err, well
under the 2e-2 gate). PSUM accumulation stays fp32; the final block's
accumulate writes bf16 staging tiles that stream to DRAM, and the host
converts back to fp32 (output quantization adds ~1e-4 to rel err).

Startup: DMA issue on the sync engine costs ~650ns per instruction and
the framework preamble ends ~6.6us in, so the first ~10 DMAs are ordered
critical-first (w1/x quarters for the k-outer warmup block) and the
warmup runs all 8 psum chains concurrently so it consumes x at only
~150GB/s while the rest streams in.
"""

import numpy as np

B, S, D, F = 4, 4096, 2048, 8192
NUM_TOKENS = 2048
NCORES = 8
ROWS = (B * NUM_TOKENS) // NCORES     # 1024 rows per core
P = 128
KT = D // P                           # 16 k-subtiles over D
FT = F // P                           # 64 f-tiles
FB = 16                               # F-blocks of 512
FSUB = 4                              # f-subtiles per block
DT = D // P                           # 16 d-tiles
NCH = ROWS // 512                     # 2 row chunks of 512
KS2 = 4                               # k-subtiles per F-block in FFN2
DG = 4                                # d-tiles per batched w2 load

_CACHE = {}


def _build():
    import concourse.bass as bass
    import concourse.mybir as mybir
    import concourse.tile as tile
    from concourse import bacc

    f32 = mybir.dt.float32
    bf16 = mybir.dt.bfloat16

    nc = bacc.Bacc()
    xt = nc.declare_dram_parameter("xt", [KT, P, ROWS], bf16, isOutput=False)
    w1 = nc.declare_dram_parameter("w1", [FT, P, KT * P], bf16, isOutput=False)
    w2 = nc.declare_dram_parameter("w2", [FB, DT // DG, P, DG * KS2 * P], bf16,
                                   isOutput=False)
    b1 = nc.declare_dram_parameter("b1", [P, FT], f32, isOutput=False)
    b2 = nc.declare_dram_parameter("b2", [P, DT], f32, isOutput=False)
    out = nc.declare_dram_parameter("out", [DT, P, ROWS], bf16, isOutput=True)

    with tile.TileContext(nc) as tc:
        with (
            tc.tile_pool(name="resident", bufs=1) as res_pool,
            tc.tile_pool(name="w1p", bufs=8) as w1p,
            tc.tile_pool(name="w2p", bufs=4) as w2p,
            tc.tile_pool(name="htp", bufs=8) as htp,
            tc.tile_pool(name="obfp", bufs=4) as obfp,
            tc.tile_pool(name="ps1", bufs=4, space="PSUM") as ps1,
            tc.tile_pool(name="ps2", bufs=4, space="PSUM") as ps2,
        ):
            xt_sb = [res_pool.tile([P, ROWS], bf16, name=f"xt{k}") for k in range(KT)]
            w1_warm = [w1p.tile([P, KT * P], bf16, name=f"w1t_{ft}", tag="w1t")
                       for ft in range(FSUB)]
            b1_sb = res_pool.tile([P, FT], f32, name="b1sb")
            b2_sb = res_pool.tile([P, DT], f32, name="b2sb")
            oacc = [res_pool.tile([P, ROWS], f32, name=f"oacc{d}") for d in range(DT)]

            # Startup DMA wave, critical-first: ~650ns issue cost each on the
            # sync engine, so the order below IS the landing order. The
            # k-outer warmup consumes (all 4 w1 quarters, xt[k]) per k at
            # ~1.7us per k; interleave so each lands just ahead of use.
            Q = 4 * P                   # quarter of a w1 tile (4 k-slices)

            def dma_w1q(ft, q):
                nc.sync.dma_start(out=w1_warm[ft][:, q * Q:(q + 1) * Q],
                                  in_=w1[ft, :, q * Q:(q + 1) * Q])

            dma_w1q(0, 0)
            nc.sync.dma_start(out=xt_sb[0][:, 0:512], in_=xt[0, :, 0:512])
            dma_w1q(1, 0)
            nc.sync.dma_start(out=xt_sb[0][:, 512:1024], in_=xt[0, :, 512:1024])
            nc.sync.dma_start(out=xt_sb[1][:], in_=xt[1])
            dma_w1q(2, 0)
            dma_w1q(3, 0)
            nc.sync.dma_start(out=xt_sb[2][:], in_=xt[2])
            nc.sync.dma_start(out=xt_sb[3][:], in_=xt[3])
            for ft in range(FSUB):
                dma_w1q(ft, 1)
            nc.sync.dma_start(out=xt_sb[4][:], in_=xt[4])
            nc.sync.dma_start(out=xt_sb[5][:], in_=xt[5])
            for ft in range(FSUB):
                dma_w1q(ft, 2)
            nc.sync.dma_start(out=xt_sb[6][:], in_=xt[6])
            nc.sync.dma_start(out=xt_sb[7][:], in_=xt[7])
            nc.sync.dma_start(out=xt_sb[8][:], in_=xt[8])
            for ft in range(FSUB):
                dma_w1q(ft, 3)
            for k in range(9, KT):
                nc.sync.dma_start(out=xt_sb[k][:], in_=xt[k])
            nc.sync.dma_start(out=b1_sb[:], in_=b1[:])
            nc.sync.dma_start(out=b2_sb[:], in_=b2[:])

            for fb in range(FB):
                ht = []
                if fb == 0:
                    # warmup block: k-outer over all 8 psum chains (4
                    # f-subtiles x 2 row chunks) so matmuls start as soon as
                    # the first w1 quarter + xt chunk land, at ~150GB/s of
                    # input demand. fs0/fs1 chains sit in ps2 so their gelu
                    # frees banks for this block's own FFN2 chains.
                    for fs in range(FSUB):
                        ht.append(htp.tile([P, ROWS], bf16, name=f"ht_{fs}", tag="ht"))
                    chains = [(fs, n) for fs in range(FSUB) for n in range(NCH)]
                    psums = {
                        (fs, n): (ps2 if fs < 2 else ps1).tile(
                            [P, 512], f32, name=f"ps1w_{fs}_{n}",
                            tag=("ps2" if fs < 2 else "ps1"))
                        for fs, n in chains
                    }
                    for k in range(KT):
                        for fs, n in chains:
                            nc.tensor.matmul(
                                psums[(fs, n)][:],
                                w1_warm[fs][:, k * P:(k + 1) * P],
                                xt_sb[k][:, n * 512:(n + 1) * 512],
                                start=(k == 0), stop=(k == KT - 1),
                            )
                    for fs, n in chains:
                        nc.scalar.activation(
                            ht[fs][:, n * 512:(n + 1) * 512], psums[(fs, n)][:],
                            mybir.ActivationFunctionType.Gelu_apprx_tanh,
                            bias=b1_sb[:, fs:fs + 1],
                        )
                else:
                    for fs in range(FSUB):
                        ft = fb * FSUB + fs
                        w1_sb = w1p.tile([P, KT * P], bf16, name=f"w1t_{ft}", tag="w1t")
                        nc.sync.dma_start(out=w1_sb[:], in_=w1[ft])
                        ht_t = htp.tile([P, ROWS], bf16, name=f"ht_{ft}", tag="ht")
                        psums = [ps1.tile([P, 512], f32, name=f"ps1_{ft}_{n}", tag="ps1")
                                 for n in range(NCH)]
                        # n innermost: consecutive matmuls share the
                        # stationary w1 k-slice.
                        for k in range(KT):
                            for n in range(NCH):
                                nc.tensor.matmul(
                                    psums[n][:],
                                    w1_sb[:, k * P:(k + 1) * P],
                                    xt_sb[k][:, n * 512:(n + 1) * 512],
                                    start=(k == 0), stop=(k == KT - 1),
                                )
                        for n in range(NCH):
                            nc.scalar.activation(
                                ht_t[:, n * 512:(n + 1) * 512], psums[n][:],
                                mybir.ActivationFunctionType.Gelu_apprx_tanh,
                                bias=b1_sb[:, ft:ft + 1],
                            )
                        ht.append(ht_t)

                for g in range(DT // DG):
                    # one batched w2 load per 4 d-tiles: DMA issue slots on
                    # the sync engine are the scarce resource, not HBM BW
                    w2_sb = w2p.tile([P, DG * KS2 * P], bf16,
                                     name=f"w2t_{fb}_{g}", tag="w2t")
                    nc.sync.dma_start(out=w2_sb[:], in_=w2[fb, g])
                    for dd in range(DG):
                        d = g * DG + dd
                        psums2 = [ps2.tile([P, 512], f32, name=f"ps2_{fb}_{d}_{n}",
                                           tag="ps2") for n in range(NCH)]
                        for ks in range(KS2):
                            for n in range(NCH):
                                nc.tensor.matmul(
                                    psums2[n][:],
                                    w2_sb[:, (dd * KS2 + ks) * P:(dd * KS2 + ks + 1) * P],
                                    ht[ks][:, n * 512:(n + 1) * 512],
                                    start=(ks == 0), stop=(ks == KS2 - 1),
                                )
                        if fb == FB - 1:
                            obf = obfp.tile([P, ROWS], bf16, name=f"obf{d}", tag="obf")
                        for n in range(NCH):
                            sl = slice(n * 512, (n + 1) * 512)
                            if fb == 0:
                                # first block: write psum + broadcast b2
                                nc.scalar.activation(
                                    oacc[d][:, sl], psums2[n][:],
                                    mybir.ActivationFunctionType.Identity,
                                    bias=b2_sb[:, d:d + 1],
                                )
                            elif fb < FB - 1:
                                nc.vector.tensor_add(
                                    oacc[d][:, sl], oacc[d][:, sl], psums2[n][:])
                            else:
                                # final block: accumulate straight into the
                                # bf16 staging tile and stream it out
                                nc.vector.tensor_add(
                                    obf[:, sl], oacc[d][:, sl], psums2[n][:])
                        if fb == FB - 1:
                            nc.sync.dma_start(out=out[d], in_=obf[:])

    nc.compile()
    return nc


def _get_nc():
    if "nc" not in _CACHE:
        _CACHE["nc"] = _build()
    return _CACHE["nc"]


def _prep_in_maps(x, W1, b1, W2, b2):
    import ml_dtypes

    bf = ml_dtypes.bfloat16
    x = np.asarray(x, dtype=np.float32)
    W1 = np.asarray(W1, dtype=np.float32)
    W2 = np.asarray(W2, dtype=np.float32)
    b1 = np.asarray(b1, dtype=np.float32)
    b2 = np.asarray(b2, dtype=np.float32)

    xs = x[:, :NUM_TOKENS, :].reshape(B * NUM_TOKENS, D)         # [8192, 2048]
    w1h = np.ascontiguousarray(
        W1.reshape(KT, P, FT, P).transpose(2, 1, 0, 3)
    ).reshape(FT, P, KT * P).astype(bf)                          # [ft, p, k*c]
    w2h = np.ascontiguousarray(
        W2.reshape(FB, KS2, P, DT // DG, DG, P).transpose(0, 3, 2, 4, 1, 5)
    ).reshape(FB, DT // DG, P, DG * KS2 * P).astype(bf)          # [fb, g, p, dd*ks*c]
    b1h = np.ascontiguousarray(b1.reshape(FT, P).T)              # [p, ft]
    b2h = np.ascontiguousarray(b2.reshape(DT, P).T)              # [p, d]

    in_maps = []
    for c in range(NCORES):
        xc = xs[c * ROWS:(c + 1) * ROWS]                         # [1024, 2048]
        xth = np.ascontiguousarray(xc.T.reshape(KT, P, ROWS)).astype(bf)
        in_maps.append({"xt": xth, "w1": w1h, "w2": w2h, "b1": b1h, "b2": b2h})
    return in_maps


def _gather(res):
    out = np.empty((B * NUM_TOKENS, D), dtype=np.float32)
    for c in range(NCORES):
        oc = np.asarray(res.results[c]["out"], dtype=np.float32)  # [d, p, n]
        out[c * ROWS:(c + 1) * ROWS] = oc.reshape(D, ROWS).T
    return out.reshape(B, NUM_TOKENS, D)


def kernel(x, Wp, bp, W1, b1, W2, b2, **_unused):
    from concourse.bass_utils import run_bass_kernel_spmd

    in_maps = _prep_in_maps(x, W1, b1, W2, b2)
    nc = _get_nc()
    res = run_bass_kernel_spmd(nc, in_maps, list(range(NCORES)))
    return _gather(res)


# revision 10
# speedup vs baseline: 1.0140x; 1.0035x over previous
"""MoD-router FFN kernel for 8 TRN2 NeuronCores (self-contained).

Math note: the reference applies softmax over a size-1 axis, which yields
all-ones scores for ANY input; jax.lax.top_k is stable, so the selected
token indices are always [0..NUM_TOKENS) per batch row. The router weights
(Wp, bp) therefore cannot affect the output, and the kernel computes

    out = gelu_tanh(x[:, :2048, :] @ W1 + b1) @ W2 + b2

Sharding: data-parallel over the 4*2048 = 8192 selected token rows ->
1024 rows per core. Each core runs a fused transposed FFN:
  H^T = gelu(W1^T @ X^T + b1)   (per F-block of 512, kept in SBUF)
  out^T += W2_blk^T @ H^T_blk   (accumulated in SBUF fp32)
Matmuls run in bf16 (same 1 col/cycle PE rate as fp32r but FWL hides
LDWEIGHTS and DMA bytes drop 2x; quantization adds ~3.4e-3 rel err, well
under the 2e-2 gate). PSUM accumulation stays fp32; the final block's
accumulate writes bf16 staging tiles that stream to DRAM, and the host
converts back to fp32 (output quantization adds ~1e-4 to rel err).

Startup: DMA issue on the sync engine costs ~650ns per instruction and
the framework preamble ends ~6.6us in, so the first ~10 DMAs are ordered
critical-first (w1/x quarters for the k-outer warmup block) and the
warmup runs all 8 psum chains concurrently so it consumes x at only
~150GB/s while the rest streams in.
"""

import numpy as np

B, S, D, F = 4, 4096, 2048, 8192
NUM_TOKENS = 2048
NCORES = 8
ROWS = (B * NUM_TOKENS) // NCORES     # 1024 rows per core
P = 128
KT = D // P                           # 16 k-subtiles over D
FT = F // P                           # 64 f-tiles
FB = 16                               # F-blocks of 512
FSUB = 4                              # f-subtiles per block
DT = D // P                           # 16 d-tiles
NCH = ROWS // 512                     # 2 row chunks of 512
KS2 = 4                               # k-subtiles per F-block in FFN2
DG = 4                                # d-tiles per batched w2 load

_CACHE = {}


def _build():
    import concourse.bass as bass
    import concourse.mybir as mybir
    import concourse.tile as tile
    from concourse import bacc

    f32 = mybir.dt.float32
    bf16 = mybir.dt.bfloat16

    nc = bacc.Bacc()
    xt = nc.declare_dram_parameter("xt", [KT, P, ROWS], bf16, isOutput=False)
    w1 = nc.declare_dram_parameter("w1", [FT, P, KT * P], bf16, isOutput=False)
    w2 = nc.declare_dram_parameter("w2", [FB, DT // DG, P, DG * KS2 * P], bf16,
                                   isOutput=False)
    b1 = nc.declare_dram_parameter("b1", [P, FT], f32, isOutput=False)
    b2 = nc.declare_dram_parameter("b2", [P, DT], f32, isOutput=False)
    out = nc.declare_dram_parameter("out", [DT, P, ROWS], bf16, isOutput=True)

    with tile.TileContext(nc) as tc:
        with (
            tc.tile_pool(name="resident", bufs=1) as res_pool,
            tc.tile_pool(name="w1p", bufs=8) as w1p,
            tc.tile_pool(name="w2p", bufs=4) as w2p,
            tc.tile_pool(name="htp", bufs=8) as htp,
            tc.tile_pool(name="obfp", bufs=4) as obfp,
            tc.tile_pool(name="ps1", bufs=4, space="PSUM") as ps1,
            tc.tile_pool(name="ps2", bufs=4, space="PSUM") as ps2,
        ):
            xt_sb = [res_pool.tile([P, ROWS], bf16, name=f"xt{k}") for k in range(KT)]
            zt = res_pool.tile([P, 512], bf16, name="zt")
            w1_warm = [w1p.tile([P, KT * P], bf16, name=f"w1t_{ft}", tag="w1t")
                       for ft in range(FSUB)]
            b1_sb = res_pool.tile([P, FT], f32, name="b1sb")
            b2_sb = res_pool.tile([P, DT], f32, name="b2sb")
            oacc = [res_pool.tile([P, ROWS], f32, name=f"oacc{d}") for d in range(DT)]

            # Startup DMA wave, critical-first: ~650ns issue cost each on the
            # sync engine, so the order below IS the landing order. The
            # k-outer warmup consumes (all 4 w1 quarters, xt[k]) per k at
            # ~1.7us per k; interleave so each lands just ahead of use.
            Q = 4 * P                   # quarter of a w1 tile (4 k-slices)

            def dma_w1q(ft, q):
                nc.sync.dma_start(out=w1_warm[ft][:, q * Q:(q + 1) * Q],
                                  in_=w1[ft, :, q * Q:(q + 1) * Q])

            dma_w1q(0, 0)
            nc.sync.dma_start(out=xt_sb[0][:, 0:512], in_=xt[0, :, 0:512])
            dma_w1q(1, 0)
            nc.sync.dma_start(out=xt_sb[0][:, 512:1024], in_=xt[0, :, 512:1024])
            nc.sync.dma_start(out=xt_sb[1][:], in_=xt[1])
            dma_w1q(2, 0)
            dma_w1q(3, 0)
            nc.sync.dma_start(out=xt_sb[2][:], in_=xt[2])
            nc.sync.dma_start(out=xt_sb[3][:], in_=xt[3])
            for ft in range(FSUB):
                dma_w1q(ft, 1)
            nc.sync.dma_start(out=xt_sb[4][:], in_=xt[4])
            nc.sync.dma_start(out=xt_sb[5][:], in_=xt[5])
            for ft in range(FSUB):
                dma_w1q(ft, 2)
            nc.sync.dma_start(out=xt_sb[6][:], in_=xt[6])
            nc.sync.dma_start(out=xt_sb[7][:], in_=xt[7])
            nc.sync.dma_start(out=xt_sb[8][:], in_=xt[8])
            for ft in range(FSUB):
                dma_w1q(ft, 3)
            for k in range(9, KT):
                nc.sync.dma_start(out=xt_sb[k][:], in_=xt[k])
            nc.sync.dma_start(out=b1_sb[:], in_=b1[:])
            nc.sync.dma_start(out=b2_sb[:], in_=b2[:])

            # HAM pre-warm: the first real matmul waits ~3.5us for its DMAs
            # while the PE sits idle and cold (K=4/8). Burn that window with
            # dummy matmuls on a memset tile so the activity monitor opens
            # the clock gate before real work arrives (~8 x 427ns cold fills
            # the gap; overshoot would delay the first real matmul).
            nc.vector.memset(zt[:], 0)
            wps = ps1.tile([P, 512], f32, name="warmps", tag="ps1")
            for i in range(8):
                nc.tensor.matmul(wps[:], zt[:, 0:P], zt[:],
                                 start=True, stop=True)

            for fb in range(FB):
                ht = []
                if fb == 0:
                    # warmup block: k-outer over all 8 psum chains (4
                    # f-subtiles x 2 row chunks) so matmuls start as soon as
                    # the first w1 quarter + xt chunk land, at ~150GB/s of
                    # input demand. fs0/fs1 chains sit in ps2 so their gelu
                    # frees banks for this block's own FFN2 chains.
                    for fs in range(FSUB):
                        ht.append(htp.tile([P, ROWS], bf16, name=f"ht_{fs}", tag="ht"))
                    chains = [(fs, n) for fs in range(FSUB) for n in range(NCH)]
                    psums = {
                        (fs, n): (ps2 if fs < 2 else ps1).tile(
                            [P, 512], f32, name=f"ps1w_{fs}_{n}",
                            tag=("ps2" if fs < 2 else "ps1"))
                        for fs, n in chains
                    }
                    # k=0 runs n-outer: the PE queue is strict FIFO and the
                    # second xt[0] chunk lands ~3us after the first, so put
                    # every n=0 chain ahead of it. k>=1 runs n-inner so
                    # consecutive matmuls share the stationary w1 slice.
                    for k in range(KT):
                        order = chains if k else sorted(chains, key=lambda c: c[1])
                        for fs, n in order:
                            nc.tensor.matmul(
                                psums[(fs, n)][:],
                                w1_warm[fs][:, k * P:(k + 1) * P],
                                xt_sb[k][:, n * 512:(n + 1) * 512],
                                start=(k == 0), stop=(k == KT - 1),
                            )
                    for fs, n in chains:
                        nc.scalar.activation(
                            ht[fs][:, n * 512:(n + 1) * 512], psums[(fs, n)][:],
                            mybir.ActivationFunctionType.Gelu_apprx_tanh,
                            bias=b1_sb[:, fs:fs + 1],
                        )
                else:
                    for fs in range(FSUB):
                        ft = fb * FSUB + fs
                        w1_sb = w1p.tile([P, KT * P], bf16, name=f"w1t_{ft}", tag="w1t")
                        nc.sync.dma_start(out=w1_sb[:], in_=w1[ft])
                        ht_t = htp.tile([P, ROWS], bf16, name=f"ht_{ft}", tag="ht")
                        psums = [ps1.tile([P, 512], f32, name=f"ps1_{ft}_{n}", tag="ps1")
                                 for n in range(NCH)]
                        # n innermost: consecutive matmuls share the
                        # stationary w1 k-slice.
                        for k in range(KT):
                            for n in range(NCH):
                                nc.tensor.matmul(
                                    psums[n][:],
                                    w1_sb[:, k * P:(k + 1) * P],
                                    xt_sb[k][:, n * 512:(n + 1) * 512],
                                    start=(k == 0), stop=(k == KT - 1),
                                )
                        for n in range(NCH):
                            nc.scalar.activation(
                                ht_t[:, n * 512:(n + 1) * 512], psums[n][:],
                                mybir.ActivationFunctionType.Gelu_apprx_tanh,
                                bias=b1_sb[:, ft:ft + 1],
                            )
                        ht.append(ht_t)

                for g in range(DT // DG):
                    # one batched w2 load per 4 d-tiles: DMA issue slots on
                    # the sync engine are the scarce resource, not HBM BW
                    w2_sb = w2p.tile([P, DG * KS2 * P], bf16,
                                     name=f"w2t_{fb}_{g}", tag="w2t")
                    nc.sync.dma_start(out=w2_sb[:], in_=w2[fb, g])
                    for dd in range(DG):
                        d = g * DG + dd
                        psums2 = [ps2.tile([P, 512], f32, name=f"ps2_{fb}_{d}_{n}",
                                           tag="ps2") for n in range(NCH)]
                        for ks in range(KS2):
                            for n in range(NCH):
                                nc.tensor.matmul(
                                    psums2[n][:],
                                    w2_sb[:, (dd * KS2 + ks) * P:(dd * KS2 + ks + 1) * P],
                                    ht[ks][:, n * 512:(n + 1) * 512],
                                    start=(ks == 0), stop=(ks == KS2 - 1),
                                )
                        if fb == FB - 1:
                            obf = obfp.tile([P, ROWS], bf16, name=f"obf{d}", tag="obf")
                        for n in range(NCH):
                            sl = slice(n * 512, (n + 1) * 512)
                            if fb == 0:
                                # first block: write psum + broadcast b2
                                nc.scalar.activation(
                                    oacc[d][:, sl], psums2[n][:],
                                    mybir.ActivationFunctionType.Identity,
                                    bias=b2_sb[:, d:d + 1],
                                )
                            elif fb < FB - 1:
                                nc.vector.tensor_add(
                                    oacc[d][:, sl], oacc[d][:, sl], psums2[n][:])
                            else:
                                # final block: accumulate straight into the
                                # bf16 staging tile and stream it out
                                nc.vector.tensor_add(
                                    obf[:, sl], oacc[d][:, sl], psums2[n][:])
                        if fb == FB - 1:
                            nc.sync.dma_start(out=out[d], in_=obf[:])

    nc.compile()
    return nc


def _get_nc():
    if "nc" not in _CACHE:
        _CACHE["nc"] = _build()
    return _CACHE["nc"]


def _prep_in_maps(x, W1, b1, W2, b2):
    import ml_dtypes

    bf = ml_dtypes.bfloat16
    x = np.asarray(x, dtype=np.float32)
    W1 = np.asarray(W1, dtype=np.float32)
    W2 = np.asarray(W2, dtype=np.float32)
    b1 = np.asarray(b1, dtype=np.float32)
    b2 = np.asarray(b2, dtype=np.float32)

    xs = x[:, :NUM_TOKENS, :].reshape(B * NUM_TOKENS, D)         # [8192, 2048]
    w1h = np.ascontiguousarray(
        W1.reshape(KT, P, FT, P).transpose(2, 1, 0, 3)
    ).reshape(FT, P, KT * P).astype(bf)                          # [ft, p, k*c]
    w2h = np.ascontiguousarray(
        W2.reshape(FB, KS2, P, DT // DG, DG, P).transpose(0, 3, 2, 4, 1, 5)
    ).reshape(FB, DT // DG, P, DG * KS2 * P).astype(bf)          # [fb, g, p, dd*ks*c]
    b1h = np.ascontiguousarray(b1.reshape(FT, P).T)              # [p, ft]
    b2h = np.ascontiguousarray(b2.reshape(DT, P).T)              # [p, d]

    in_maps = []
    for c in range(NCORES):
        xc = xs[c * ROWS:(c + 1) * ROWS]                         # [1024, 2048]
        xth = np.ascontiguousarray(xc.T.reshape(KT, P, ROWS)).astype(bf)
        in_maps.append({"xt": xth, "w1": w1h, "w2": w2h, "b1": b1h, "b2": b2h})
    return in_maps


def _gather(res):
    out = np.empty((B * NUM_TOKENS, D), dtype=np.float32)
    for c in range(NCORES):
        oc = np.asarray(res.results[c]["out"], dtype=np.float32)  # [d, p, n]
        out[c * ROWS:(c + 1) * ROWS] = oc.reshape(D, ROWS).T
    return out.reshape(B, NUM_TOKENS, D)


def kernel(x, Wp, bp, W1, b1, W2, b2, **_unused):
    from concourse.bass_utils import run_bass_kernel_spmd

    in_maps = _prep_in_maps(x, W1, b1, W2, b2)
    nc = _get_nc()
    res = run_bass_kernel_spmd(nc, in_maps, list(range(NCORES)))
    return _gather(res)
